# revision 46
# baseline (speedup 1.0000x reference)
"""Distributed Trainium2 kernel for nn_Attention_61332132987140.

Gated multi-head attention block: RMSNorm -> QKV proj -> RoPE -> softmax
attention -> sigmoid head gating -> output projection.

Sharding: 8 cores = 2 batch groups x 4-head groups (tensor parallel on
heads). Each core computes attention for its batch's full sequence over
its 4 heads and the partial output projection; bf16 ReduceScatters over
each 4-core batch group sum the partials, writing the external output
directly. The host reassembles the full (2, 2048, 1024) output.

Cost-model-driven structure:
- activations arrive host-transposed (xT): no on-device xn transposes
- attention-V matmuls in [query, dim] orientation: full 128 output
  partitions per instruction (half the charge of [dim, query])
- softmax denominators as free-size-1 matmuls: ~130k PE cycles saved
- gates ride as 4 extra columns of the V projection; sigmoid via exp
- RMSNorm scale: ACT Square+accumulate for sum-sq, quake-rsqrt Newton on
  DVE (no Ln/Exp act-table switching), folded into RoPE via fused
  scalar_tensor_tensor (t = (qk*s)*cos in one DVE op)
- softmax exp 3-way split over ACT (true exp) / Pool+DVE (Schraudolph
  bf16-bits: bits = s*scale*log2e*128 + B, truncated to uint16, bitcast
  to bf16). Pool is the cheapest PSUM reader (no access penalty, no
  dtype throughput loss), ACT/DVE/Pool loads balanced ~16us/quarter
  each under PE's 24us/quarter.
- PSUM accumulator banks (av/sums) hold several col-disjoint groups, so
  they are zeroed by K=1 zero matmuls and accumulated with start=False
  (start=True would mark the whole bank pending-zero and wipe siblings)
- per-quarter tail split in two: normalize+transpose emitted at the next
  quarter's start (frees psum bufs early), out-projection one jt-pair
  later (keeps PE streaming scores across the boundary)
- one ReduceScatter per 512-query quarter issued as soon as that
  quarter's y hits DRAM (RS costs 15us fixed + bytes/40GBps and doesn't
  block the issuing engine); the last quarter is split into two 256-row
  halves so only a ~18us collective trails the final compute. RS writes
  the external output directly (no SBUF bounce).
- DMA queues balanced: xb/cos on SP, xT/sin/wqkv[0:4] on Pool,
  wqkv[4:8] on ACT (ahead of the squares), wout on SP.
"""
import os
import sys

sys.path.insert(0, "/opt/trn_rl_repo")

import numpy as np
import ml_dtypes

import concourse.bass as bass
import concourse.mybir as mybir
import concourse.tile as tile
from concourse import bacc
from concourse.bass_utils import run_bass_kernel_spmd

F32 = mybir.dt.float32
BF16 = mybir.dt.bfloat16
I32 = mybir.dt.int32
U16 = mybir.dt.uint16
AF = mybir.ActivationFunctionType
ALU = mybir.AluOpType

B, N, DIM = 2, 2048, 1024
HEADS, DH = 16, 64
HL = 4  # local heads per core
P = 128
TT = N // P  # 16 token tiles
KD = DIM // P  # 8 contraction tiles
NQ = 4  # quarters (512-query chunks)
CORES = 8
REPLICA_GROUPS = [[0, 1, 2, 3], [4, 5, 6, 7]]

# exp engine schedule: A=ACT true exp, D=DVE Schraudolph. Pool cannot read
# PSUM (BIR verifier: "GPSIMD Instructions cannot access PSUM"), so the exp
# is split across the only two PSUM-capable elementwise engines. DVE's share
# is front-loaded into m0-5 and the last two m's of each quarter are all-ACT,
# so the DVE queue is drained at quarter boundaries: tail_pre (recips + o_sb
# on DVE) then frees the av/sums psum banks immediately instead of ~9us late.
EXP_PAT_M0 = ['A', 'D', 'A', 'A', 'D', 'A', 'A', 'A']     # m0: DVE busy w/ tail_pre
EXP_PAT_EARLY = ['D', 'A', 'D', 'D', 'A', 'D', 'D', 'A']  # m 1..5: 5 DVE
EXP_PAT_LATE = ['D', 'A', 'D', 'A', 'A', 'A', 'A', 'A']   # m 6..7: 2 DVE, early

# ReduceScatter grouping: each collective freezes ALL DMA traffic for its
# full duration in the cost model (no DMA slice ever overlaps a collective),
# so fewer/larger collectives beat per-quarter ones despite the bigger tail.
# Groups must be consecutive quarters; a SINGLE end-of-kernel RS measured
# best (212.4us vs 215.9-221.1 for every split) since any mid-kernel
# collective freezes the XBAR transposes + ydram stores of the quarter it
# overlaps.
RS_GROUPS = [[int(c) for c in grp] for grp in
             os.environ.get("KRSG", "0123").split(",")]

_nc_cache = None
_last_result = None


def _build():
    nc = bacc.Bacc("TRN2", target_bir_lowering=False, debug=False, num_devices=CORES)

    xT_ext = nc.declare_dram_parameter("xT", [DIM, N], BF16, isOutput=False)
    xb_ext = nc.declare_dram_parameter("xb", [N, DIM], BF16, isOutput=False)
    wqkv_ext = nc.declare_dram_parameter("wqkv", [DIM, 772], BF16, isOutput=False)
    wout_ext = nc.declare_dram_parameter("wout", [2 * P, DIM], BF16, isOutput=False)
    cosP_ext = nc.declare_dram_parameter("cosP", [N, 512], BF16, isOutput=False)
    sinN_ext = nc.declare_dram_parameter("sinN", [N, 512], BF16, isOutput=False)
    bgn_ext = nc.declare_dram_parameter("bgn", [P, HL], F32, isOutput=False)
    out_ext = nc.declare_dram_parameter("out", [4 * P, DIM], BF16, isOutput=True)

    with tile.TileContext(nc) as tc:
        with (
            tc.tile_pool(name="wpool", bufs=1) as wpool,
            tc.tile_pool(name="persist", bufs=1) as persist,
            tc.tile_pool(name="xbp", bufs=8) as xbp,
            tc.tile_pool(name="sqp", bufs=4) as sqp,
            tc.tile_pool(name="small", bufs=8) as small,
            tc.tile_pool(name="tup", bufs=6) as tup,
            tc.tile_pool(name="qksp", bufs=4) as qksp,
            tc.tile_pool(name="ptp", bufs=14) as ptp,
            tc.tile_pool(name="op", bufs=2) as op_pool,
            tc.tile_pool(name="otp", bufs=2) as otp,
            tc.tile_pool(name="ysq", bufs=2) as ysq,
            tc.tile_pool(name="obp", bufs=2) as obp,
            # one 5-slot pool for all transient [128,512] psum tiles
            # (phase-A qk, phase-B scores s_t, tail out-proj y_ps): the
            # score->exp->free round trip (~1.4us) across 5 slots keeps PE
            # ahead of the exp pipeline; 3 slots paced the whole m-loop
            tc.tile_pool(name="ps_big", bufs=5, space="PSUM") as ps_big,
            tc.tile_pool(name="ps_va", bufs=2, space="PSUM") as ps_va,
            tc.tile_pool(name="dram", bufs=1, space="DRAM") as dramp,
        ):
            # ---- constants / weights (loads emitted later, ordered for
            # startup: xT/wqkv split across Pool+ACT, xb on SP) ----
            wqkv_sb = wpool.tile([P, KD, 772], BF16)
            wout_sb = wpool.tile([P, 2, DIM], BF16)
            bgn_sb = wpool.tile([P, HL], F32)
            cosP_sb = wpool.tile([P, TT, 512], BF16)
            sinN_sb = wpool.tile([P, TT, 512], BF16)
            zb = wpool.tile([P, 1], F32)
            nc.vector.memset(zb[:], 0.0)
            ones_mm = wpool.tile([P, 1], BF16)
            nc.vector.memset(ones_mm[:], 1.0)
            ones1 = wpool.tile([1, P], BF16)
            nc.vector.memset(ones1[:], 1.0)
            zrow = wpool.tile([1, 512], BF16)
            nc.vector.memset(zrow[:], 0.0)

            # ---- persistent activations ----
            xT_sb = persist.tile([P, KD, N], BF16)
            # qkT blocks: 0=q(h0,h1) 1=q(h2,h3) 2=k(h0,h1) 3=k(h2,h3); rows=dh
            qkT_sb = persist.tile([P, 4, N], BF16)
            v_sb = persist.tile([P, TT, 256], BF16)
            gates_sb = persist.tile([P, TT, HL], F32)
            ss_all = persist.tile([P, TT], F32)
            s_all = persist.tile([P, TT], F32)

            def emit_norm_group(g):
                """sum-sq (ACT Square+accum) + quake rsqrt for tiles 4g..4g+3.

                s = 32*ss^-0.5 with the /1024 folded into the quake seed's
                pre-scale; two Newton iterations on DVE. No act-table loads.
                """
                for tt in range(4 * g, 4 * g + 4):
                    xb_t = xbp.tile([P, DIM], BF16, name="xb_t")
                    nc.sync.dma_start(xb_t[:], xb_ext[tt * P:(tt + 1) * P, :])
                    scr = sqp.tile([P, DIM], BF16, name="scr")
                    nc.scalar.activation(scr[:], xb_t[:], AF.Square,
                                         accum_out=ss_all[:, tt:tt + 1])
                # quake-rsqrt Newton on DVE (Pool only supports TensorTensor/DMA)
                gs = slice(4 * g, 4 * g + 4)
                xp = small.tile([P, 4], F32, name="xp")
                nc.vector.tensor_scalar(out=xp[:], in0=ss_all[:, gs],
                                        scalar1=1.0 / 1024.0, scalar2=None,
                                        op0=ALU.mult)
                jt_ = small.tile([P, 4], I32, name="jt_")
                nc.vector.tensor_scalar(out=jt_[:], in0=xp[:].bitcast(I32),
                                        scalar1=1, scalar2=None,
                                        op0=ALU.logical_shift_right)
                y0b = small.tile([P, 4], I32, name="y0b")
                nc.vector.tensor_scalar(out=y0b[:], in0=jt_[:], scalar1=-1,
                                        scalar2=0x5f3759df, op0=ALU.mult,
                                        op1=ALU.add)
                cur = y0b[:].bitcast(F32)
                for it in range(2):
                    h_ = small.tile([P, 4], F32, name="h_")
                    nc.vector.tensor_tensor(out=h_[:], in0=cur, in1=cur,
                                            op=ALU.mult)
                    hx = small.tile([P, 4], F32, name="hx")
                    nc.vector.tensor_tensor(out=hx[:], in0=h_[:], in1=xp[:],
                                            op=ALU.mult)
                    w_ = small.tile([P, 4], F32, name="w_")
                    nc.vector.tensor_scalar(out=w_[:], in0=hx[:], scalar1=-0.5,
                                            scalar2=1.5, op0=ALU.mult,
                                            op1=ALU.add)
                    dst = s_all[:, gs] if it == 1 else \
                        small.tile([P, 4], F32, name="nx")[:]
                    nc.vector.tensor_tensor(out=dst, in0=cur, in1=w_[:],
                                            op=ALU.mult)
                    cur = dst

            def emit_tables_group(g):
                gs = slice(4 * g, 4 * g + 4)
                nc.sync.dma_start(
                    cosP_sb[:, gs, :],
                    cosP_ext.rearrange("(t p) f -> p t f", p=P)[:, gs, :])
                nc.gpsimd.dma_start(
                    sinN_sb[:, gs, :],
                    sinN_ext.rearrange("(t p) f -> p t f", p=P)[:, gs, :])

            # startup order: first xT sliver + wqkv halves (Pool/ACT, so the
            # ACT chunk lands before the squares), norm group 0 (xb on SP),
            # remaining xT, tables group 0, then per-group interleave
            xT_re = xT_ext.rearrange("(k p) t -> p k t", p=P)
            wq_re = wqkv_ext.rearrange("(k p) f -> p k f", p=P)
            nc.gpsimd.dma_start(xT_sb[:, :, 0:128], xT_re[:, :, 0:128])
            nc.gpsimd.dma_start(wqkv_sb[:, 0:2, :], wq_re[:, 0:2, :])
            nc.sync.dma_start(wqkv_sb[:, 2:4, :], wq_re[:, 2:4, :])
            nc.scalar.dma_start(wqkv_sb[:, 4:8, :], wq_re[:, 4:8, :])
            emit_norm_group(0)
            nc.gpsimd.dma_start(xT_sb[:, :, 128:512], xT_re[:, :, 128:512])
            emit_tables_group(0)
            nc.scalar.dma_start(bgn_sb[:], bgn_ext[:])
            # prefetch norm group 1 right away: rope(t4) needs s(g1), which
            # needs 4 squares serialized on ACT — a group emitted only at
            # tt=4 arrives ~5us late and stalls the qk psum slots
            emit_norm_group(1)
            emit_tables_group(1)
            for c in range(1, 4):
                nc.gpsimd.dma_start(
                    xT_sb[:, :, c * 512:(c + 1) * 512], xT_re[:, :, c * 512:(c + 1) * 512])

            # ---- phase A per token tile: QKV + RoPE + transposes + gates ----
            for tt in range(TT):
                if tt % 4 == 0 and 4 <= tt <= 8:
                    emit_norm_group(tt // 4 + 1)
                    emit_tables_group(tt // 4 + 1)
                s_ap = s_all[:, tt:tt + 1]
                # qk_ps gets the 3-slot pool (long rope-read lifetime), vg the
                # 2-slot one; both pools are otherwise idle during phase A
                qk_ps = ps_big.tile([P, 512], F32, name="qk_ps", tag="big")
                vg_ps = ps_va.tile([P, 260], F32, name="vg_ps", tag="va")
                for kd in range(KD):
                    lhsT = xT_sb[:, kd, tt * P:(tt + 1) * P]
                    nc.tensor.matmul(qk_ps[:], lhsT, wqkv_sb[:, kd, 0:512],
                                     start=(kd == 0), stop=(kd == KD - 1))
                    nc.tensor.matmul(vg_ps[:], lhsT, wqkv_sb[:, kd, 512:772],
                                     start=(kd == 0), stop=(kd == KD - 1))

                # rope with norm scale fused: qk' = (qk*s)*cos + (swap*s)*sin
                t_sb = tup.tile([P, 512], BF16, name="t_sb")
                nc.vector.scalar_tensor_tensor(
                    out=t_sb[:], in0=qk_ps[:], scalar=s_ap,
                    in1=cosP_sb[:, tt, :], op0=ALU.mult, op1=ALU.mult)
                u_sb = tup.tile([P, 512], BF16, name="u_sb")
                qkv8 = qk_ps[:].rearrange("p (b c) -> p b c", b=8)
                u8 = u_sb[:].rearrange("p (b c) -> p b c", b=8)
                sin8 = sinN_sb[:, tt, :].rearrange("p (b c) -> p b c", b=8)
                nc.vector.scalar_tensor_tensor(
                    out=u8[:, :, 0:32], in0=qkv8[:, :, 32:64], scalar=s_ap,
                    in1=sin8[:, :, 0:32], op0=ALU.mult, op1=ALU.mult)
                nc.vector.scalar_tensor_tensor(
                    out=u8[:, :, 32:64], in0=qkv8[:, :, 0:32], scalar=s_ap,
                    in1=sin8[:, :, 32:64], op0=ALU.mult, op1=ALU.mult)
                qk_sb = qksp.tile([P, 512], BF16, name="qk_sb")
                nc.gpsimd.tensor_tensor(out=qk_sb[:], in0=t_sb[:], in1=u_sb[:],
                                        op=ALU.add)
                nc.sync.dma_start_transpose(qkT_sb[:, :, tt * P:(tt + 1) * P], qk_sb[:])

                # v with norm scale (PSUM read: DVE)
                nc.vector.tensor_scalar(out=v_sb[:, tt, :], in0=vg_ps[:, 0:256],
                                        scalar1=s_ap, scalar2=None, op0=ALU.mult)

                # gates: sigmoid(s*z + b) via exp
                zt = small.tile([P, HL], BF16, name="zt")
                nc.vector.scalar_tensor_tensor(out=zt[:], in0=vg_ps[:, 256:260],
                                               scalar=s_ap, in1=bgn_sb[:],
                                               op0=ALU.mult, op1=ALU.add)
                ge = small.tile([P, HL], F32, name="ge")
                nc.scalar.activation(ge[:], zt[:], AF.Exp, scale=-1.0, bias=zb[:])
                gp = small.tile([P, HL], F32, name="gp")
                nc.vector.tensor_scalar_add(gp[:], ge[:], 1.0)
                nc.vector.reciprocal(gates_sb[:, tt, :], gp[:])

            nc.sync.dma_start(wout_sb[:], wout_ext.rearrange("(k p) f -> p k f", p=P))

            # ---- phase B: attention + out proj + per-quarter RS ----
            ydram_all = dramp.tile([4 * 512, DIM], BF16, name="ydall", tag="ydall")
            grp_obase = [sum(len(q) * P for q in RS_GROUPS[:i])
                         for i in range(len(RS_GROUPS))]
            rsout_g = [dramp.tile([len(Q) * P, DIM], BF16, name=f"rsg{i}",
                                  tag=f"rsg{i}")
                       for i, Q in enumerate(RS_GROUPS)]
            rs_after = {Q[-1]: gi for gi, Q in enumerate(RS_GROUPS)}

            def emit_rs(gi):
                Q = RS_GROUPS[gi]
                r0, r1 = Q[0] * 512, (Q[-1] + 1) * 512
                if not os.environ.get("KNOCOLL"):
                    nc.gpsimd.collective_compute(
                        "ReduceScatter", ALU.add,
                        replica_groups=REPLICA_GROUPS,
                        ins=[ydram_all[r0:r1, :].opt()],
                        outs=[rsout_g[gi][:].opt()],
                    )
                else:
                    nc.gpsimd.dma_start(
                        out_ext[grp_obase[gi]:grp_obase[gi] + len(Q) * P, :],
                        ydram_all[r0:r0 + len(Q) * P, :])

            # Schraudolph exp-to-bf16 bits: bits = s*0.125*log2e*128 + B
            SCH_A = float(0.125 * np.log2(np.e) * 128.0)
            SCH_B = float(16256.5 - 5.5)

            def emit_scores(ci, m):
                """scores+exp for jt pair (2m, 2m+1), all 4 heads."""
                pts = []
                for h in range(4):
                    pt = ptp.tile([P, 2, 512], BF16, name="pt")
                    for par in range(2):
                        jt = 2 * m + par
                        hh = 64 * (h % 2)
                        s_t = ps_big.tile([P, 512], F32, name="s_t", tag="big")
                        nc.tensor.matmul(
                            s_t[:],
                            qkT_sb[hh:hh + 64, 2 + h // 2, jt * P:(jt + 1) * P],
                            qkT_sb[hh:hh + 64, h // 2, ci * 512:(ci + 1) * 512],
                            start=True, stop=True,
                        )
                        pat = EXP_PAT_M0 if m == 0 else (
                            EXP_PAT_EARLY if m < 6 else EXP_PAT_LATE)
                        eng = pat[h * 2 + par]
                        if eng == 'A':
                            nc.scalar.activation(pt[:, par, :], s_t[:], AF.Exp,
                                                 scale=0.125, bias=zb[:])
                        else:
                            nc.vector.tensor_scalar(
                                out=pt[:, par, :].bitcast(U16),
                                in0=s_t[:], scalar1=SCH_A, scalar2=SCH_B,
                                op0=ALU.mult, op1=ALU.add)
                    pts.append(pt)
                return pts

            def emit_av(m, pts, av_t, sums_t):
                for h in range(4):
                    for qt in (0, 2, 1, 3):  # alternate psum banks
                        for par in range(2):
                            jt = 2 * m + par
                            nc.tensor.matmul(
                                av_t[qt][:, h * DH:(h + 1) * DH],
                                pts[h][:, par, qt * P:(qt + 1) * P],
                                v_sb[:, jt, h * DH:(h + 1) * DH],
                                start=False, stop=(m == TT // 2 - 1 and par == 1),
                                skip_group_check=True,
                            )
                            nc.tensor.matmul(
                                sums_t[:, (h * 4 + qt):(h * 4 + qt) + 1],
                                pts[h][:, par, qt * P:(qt + 1) * P],
                                ones_mm[:, 0:1],
                                start=False, stop=(m == TT // 2 - 1 and par == 1),
                                skip_group_check=True,
                            )

            def emit_tail_pre(ci, av_pair, sums_t):
                """normalize + gate + transpose; frees av/sums psum bufs.

                All four recips come first so the sums bank frees after
                ~0.8us (the next quarter's zero-matmul waits on it)."""
                av_t = [av_pair[qt // 2][:, qt % 2, :] for qt in range(4)]
                oT = otp.tile([P, 2, 512], BF16, name="oT")
                recs = []
                for qt in range(4):
                    rec = small.tile([P, HL], F32, name="rec")
                    nc.vector.reciprocal(
                        rec[:],
                        sums_t[:, 0:16].rearrange("p (h q) -> p q h", q=4)[:, qt, :])
                    recs.append(rec)
                for qt in range(4):
                    scl = small.tile([P, HL], F32, name="scl")
                    nc.vector.tensor_tensor(out=scl[:], in0=recs[qt][:],
                                            in1=gates_sb[:, ci * 4 + qt, :],
                                            op=ALU.mult)
                    o_sb = op_pool.tile([P, 256], BF16, name="o_sb")
                    scl_b = bass.AP(scl.tensor, scl.offset,
                                    [scl.ap[0], [1, 4], [0, DH]])
                    nc.vector.tensor_tensor(
                        out=o_sb[:].rearrange("p (h d) -> p h d", h=4),
                        in0=av_t[qt][:].rearrange("p (h d) -> p h d", h=4),
                        in1=scl_b, op=ALU.mult)
                    nc.sync.dma_start_transpose(oT[:, :, qt * P:(qt + 1) * P],
                                                o_sb[:])
                return oT

            def emit_y_qt(qt, oT, y_sbq):
                """out-projection + psum->sbuf copy for one 128-query block."""
                for oh in range(2):
                    y_ps = ps_big.tile([P, 512], F32, name="y_ps", tag="big")
                    for kh in range(2):
                        nc.tensor.matmul(
                            y_ps[:],
                            oT[:, kh, qt * P:(qt + 1) * P],
                            wout_sb[:, kh, oh * 512:(oh + 1) * 512],
                            start=(kh == 0), stop=(kh == 1),
                        )
                    dst = y_sbq[:, qt, oh * 512:(oh + 1) * 512]
                    if (qt + oh) % 2 == 0:
                        nc.scalar.activation(dst, y_ps[:], AF.Copy, bias=0.0)
                    else:
                        nc.vector.tensor_copy(dst, y_ps[:])

            def emit_tail_y_half(ci, oT, y_sbq, half):
                """out-projection + ydram store for 2 of 4 query blocks;
                splitting across two m's halves the y_ps psum-slot pressure
                that otherwise starves the score->exp pipeline mid-quarter.
                The group RS fires after the second half of its last
                quarter."""
                for qt in (2 * half, 2 * half + 1):
                    emit_y_qt(qt, oT, y_sbq)
                nc.sync.dma_start(
                    ydram_all[ci * 512:(ci + 1) * 512, :]
                    .rearrange("(q p) o -> p q o", p=P)
                    [:, 2 * half:2 * half + 2, :],
                    y_sbq[:, 2 * half:2 * half + 2, :])
                if half == 1 and ci in rs_after:
                    emit_rs(rs_after[ci])

            def emit_bounce(gi):
                """rsout -> out via SBUF, emitted at the very end of the
                program so its RS-completion wait never head-blocks a queue
                that still has time-critical work."""
                if os.environ.get("KNOCOLL"):
                    return
                Q = RS_GROUPS[gi]
                engs = [nc.gpsimd, nc.sync, nc.scalar]
                for k in range(len(Q)):
                    # rotate queues so the post-RS copies pipeline 3-wide
                    e = engs[k % 3]
                    bn = obp.tile([P, DIM], BF16, name="bn")
                    e.dma_start(bn[:], rsout_g[gi][k * P:(k + 1) * P, :])
                    e.dma_start(
                        out_ext[grp_obase[gi] + k * P:grp_obase[gi] + (k + 1) * P, :],
                        bn[:])

            def emit_tail_final(ci, av_pair, sums_t):
                """final quarter: per-qt pipeline (normalize -> transpose ->
                out-proj -> per-qt ydram store) so PE restarts ~1.5us after
                the last AV instead of waiting for the whole tail_pre, then
                one RS for the quarter."""
                av_t = [av_pair[qt // 2][:, qt % 2, :] for qt in range(4)]
                oT = otp.tile([P, 2, 512], BF16, name="oT")
                y_sbq = ysq.tile([P, 4, DIM], BF16, name="y_sbq")
                yd_re = ydram_all[ci * 512:(ci + 1) * 512, :].rearrange(
                    "(q p) o -> p q o", p=P)
                recs = []
                for qt in range(4):
                    rec = small.tile([P, HL], F32, name="rec")
                    nc.vector.reciprocal(
                        rec[:],
                        sums_t[:, 0:16].rearrange("p (h q) -> p q h", q=4)[:, qt, :])
                    recs.append(rec)
                for qt in range(4):
                    scl = small.tile([P, HL], F32, name="scl")
                    nc.vector.tensor_tensor(out=scl[:], in0=recs[qt][:],
                                            in1=gates_sb[:, ci * 4 + qt, :],
                                            op=ALU.mult)
                    o_sb = op_pool.tile([P, 256], BF16, name="o_sb")
                    scl_b = bass.AP(scl.tensor, scl.offset,
                                    [scl.ap[0], [1, 4], [0, DH]])
                    nc.vector.tensor_tensor(
                        out=o_sb[:].rearrange("p (h d) -> p h d", h=4),
                        in0=av_t[qt][:].rearrange("p (h d) -> p h d", h=4),
                        in1=scl_b, op=ALU.mult)
                    nc.sync.dma_start_transpose(oT[:, :, qt * P:(qt + 1) * P],
                                                o_sb[:])
                    emit_y_qt(qt, oT, y_sbq)
                    e = nc.sync if qt % 2 == 0 else nc.gpsimd
                    e.dma_start(yd_re[:, qt, :], y_sbq[:, qt, :])
                emit_rs(rs_after[ci])

            pending = None  # (ci, av_pair, sums_t) awaiting tail emission
            bounce_pending = None  # quarter whose RS is in flight, bounce due
            oT_prev = None
            for ci in range(NQ):
                if pending is not None:
                    oT_prev = emit_tail_pre(*pending)
                av_pair = [ps_va.tile([P, 2, 256], F32, name=f"av{i}", tag="va")
                           for i in range(2)]
                av_t = [av_pair[qt // 2][:, qt % 2, :] for qt in range(4)]
                sums_t = ps_big.tile([P, 16], F32, name="sums_t", tag="sums", bufs=1)
                # AV lags two m's behind scores: av(m) is emitted after
                # scores(m+2), so every pt has ~2 m-periods (~6us) of exp
                # slack and PE never waits on stragglers, even across the
                # quarter boundary where tail_pre occupies DVE first.
                pts_q = []
                for m in range(TT // 2):
                    pts_q.append(emit_scores(ci, m))
                    if m == 1:
                        # zero accumulator banks via K=1 zero matmuls (see
                        # docstring); emitted ~2 m-periods into the quarter
                        # so tail_pre has long freed the previous banks
                        for i in range(2):
                            nc.tensor.matmul(av_pair[i][:], ones1[:], zrow[:],
                                             start=True, stop=True)
                        nc.tensor.matmul(sums_t[:], ones1[:], zrow[:, 0:16],
                                         start=True, stop=True)
                    if m >= 2:
                        emit_av(m - 2, pts_q[m - 2], av_t, sums_t)
                    if m == 1 and pending is not None:
                        y_sbq_prev = ysq.tile([P, 4, DIM], BF16, name="y_sbq")
                        emit_tail_y_half(pending[0], oT_prev, y_sbq_prev, 0)
                    if m == 3 and pending is not None:
                        emit_tail_y_half(pending[0], oT_prev, y_sbq_prev, 1)
                        bounce_pending = pending[0]
                        pending = None
                emit_av(TT // 2 - 2, pts_q[TT // 2 - 2], av_t, sums_t)
                emit_av(TT // 2 - 1, pts_q[TT // 2 - 1], av_t, sums_t)
                pending = (ci, av_pair, sums_t)

            emit_tail_final(*pending)
            for gi in range(len(RS_GROUPS)):
                emit_bounce(gi)

    nc.compile()
    return nc


def _get_nc():
    global _nc_cache
    if _nc_cache is None:
        _nc_cache = _build()
    return _nc_cache


_PERM_EO = np.concatenate([np.arange(0, DH, 2), np.arange(1, DH, 2)])


def _shard(core, x, rotary_cos, rotary_sin, gamma, w_qkv, w_gates, b_gates, w_out):
    g, r = core // 4, core % 4
    heads = np.arange(4 * r, 4 * r + 4)
    wq = w_qkv[0 * DIM:1 * DIM] * gamma[None, :]
    wk = w_qkv[1 * DIM:2 * DIM] * gamma[None, :]
    wv = w_qkv[2 * DIM:3 * DIM]

    def qk_rows(w):
        idx = (heads[:, None] * DH + _PERM_EO[None, :]).reshape(-1)
        return w[idx]

    v_rows = wv[(heads[:, None] * DH + np.arange(DH)[None, :]).reshape(-1)]
    wg_rows = w_gates[heads] * gamma[None, :]
    wqkv_t = np.concatenate([qk_rows(wq), qk_rows(wk), v_rows, wg_rows],
                            axis=0).T
    wout_t = w_out[:, heads[0] * DH:heads[0] * DH + HL * DH].T

    cos = rotary_cos[0, 0]  # (N, DH)
    sin = rotary_sin[0, 0]
    cosP = np.tile(np.concatenate([cos[:, 0::2], cos[:, 1::2]], axis=1), (1, 8))
    sinN = np.tile(np.concatenate([-sin[:, 0::2], sin[:, 1::2]], axis=1), (1, 8))

    bf = ml_dtypes.bfloat16
    return {
        "xT": np.ascontiguousarray(x[g].T).astype(bf),
        "xb": np.ascontiguousarray(x[g]).astype(bf),
        "wqkv": np.ascontiguousarray(wqkv_t).astype(bf),
        "wout": np.ascontiguousarray(wout_t).astype(bf),
        "cosP": np.ascontiguousarray(cosP).astype(bf),
        "sinN": np.ascontiguousarray(sinN).astype(bf),
        "bgn": np.tile(b_gates[heads][None, :], (P, 1)).astype(np.float32),
    }


def kernel(x, rotary_cos, rotary_sin, gamma, w_qkv, w_gates, b_gates, w_out):
    global _last_result
    args = [np.asarray(a, np.float32) for a in
            (x, rotary_cos, rotary_sin, gamma, w_qkv, w_gates, b_gates, w_out)]
    nc = _get_nc()
    in_maps = [_shard(c, *args) for c in range(CORES)]
    try:
        res = run_bass_kernel_spmd(
            nc, in_maps, core_ids=list(range(CORES)),
            trace=bool(os.environ.get("KTRACE")),
        )
    except ModuleNotFoundError:
        res = run_bass_kernel_spmd(nc, in_maps, core_ids=list(range(CORES)))
    _last_result = res
    full = np.zeros((B, N, DIM), np.float32)
    for c in range(CORES):
        g, r = c // 4, c % 4
        o = np.asarray(res.results[c]["out"]).astype(np.float32)
        base = 0
        for Q in RS_GROUPS:
            ow = len(Q) * P
            qs = Q[0] * 512
            full[g, qs + r * ow:qs + (r + 1) * ow, :] = o[base:base + ow]
            base += ow
    return full


# revision 47
# speedup vs baseline: 1.0714x; 1.0714x over previous
"""Distributed Trainium2 kernel for nn_Attention_61332132987140.

Gated multi-head attention block: RMSNorm -> QKV proj -> RoPE -> softmax
attention -> sigmoid head gating -> output projection.

Sharding: 8 cores = 2 batch groups x 4-head groups (tensor parallel on
heads). Each core computes attention for its batch's full sequence over
its 4 heads and the partial output projection; bf16 ReduceScatters over
each 4-core batch group sum the partials, writing the external output
directly. The host reassembles the full (2, 2048, 1024) output.

Cost-model-driven structure:
- activations arrive host-transposed (xT): no on-device xn transposes
- attention-V matmuls in [query, dim] orientation: full 128 output
  partitions per instruction (half the charge of [dim, query])
- softmax denominators as free-size-1 matmuls: ~130k PE cycles saved
- gates ride as 4 extra columns of the V projection; sigmoid via exp
- RMSNorm scale: ACT Square+accumulate for sum-sq, quake-rsqrt Newton on
  DVE (no Ln/Exp act-table switching), folded into RoPE via fused
  scalar_tensor_tensor (t = (qk*s)*cos in one DVE op)
- softmax exp 3-way split over ACT (true exp) / Pool+DVE (Schraudolph
  bf16-bits: bits = s*scale*log2e*128 + B, truncated to uint16, bitcast
  to bf16). Pool is the cheapest PSUM reader (no access penalty, no
  dtype throughput loss), ACT/DVE/Pool loads balanced ~16us/quarter
  each under PE's 24us/quarter.
- PSUM accumulator banks (av/sums) hold several col-disjoint groups, so
  they are zeroed by K=1 zero matmuls and accumulated with start=False
  (start=True would mark the whole bank pending-zero and wipe siblings)
- per-quarter tail split in two: normalize+transpose emitted at the next
  quarter's start (frees psum bufs early), out-projection one jt-pair
  later (keeps PE streaming scores across the boundary)
- one ReduceScatter per 512-query quarter issued as soon as that
  quarter's y hits DRAM (RS costs 15us fixed + bytes/40GBps and doesn't
  block the issuing engine); the last quarter is split into two 256-row
  halves so only a ~18us collective trails the final compute. RS writes
  the external output directly (no SBUF bounce).
- DMA queues balanced: xb/cos on SP, xT/sin/wqkv[0:4] on Pool,
  wqkv[4:8] on ACT (ahead of the squares), wout on SP.
"""
import os
import sys

sys.path.insert(0, "/opt/trn_rl_repo")

import numpy as np
import ml_dtypes

import concourse.bass as bass
import concourse.mybir as mybir
import concourse.tile as tile
from concourse import bacc
from concourse.bass_utils import run_bass_kernel_spmd

F32 = mybir.dt.float32
BF16 = mybir.dt.bfloat16
I32 = mybir.dt.int32
U16 = mybir.dt.uint16
AF = mybir.ActivationFunctionType
ALU = mybir.AluOpType

B, N, DIM = 2, 2048, 1024
HEADS, DH = 16, 64
HL = 4  # local heads per core
P = 128
TT = N // P  # 16 token tiles
KD = DIM // P  # 8 contraction tiles
NQ = 4  # quarters (512-query chunks)
CORES = 8
REPLICA_GROUPS = [[0, 1, 2, 3], [4, 5, 6, 7]]

# exp engine schedule: A=ACT true exp, D=DVE Schraudolph. Pool cannot read
# PSUM (BIR verifier: "GPSIMD Instructions cannot access PSUM"), so the exp
# is split across the only two PSUM-capable elementwise engines. DVE's share
# is front-loaded into m0-5 and the last two m's of each quarter are all-ACT,
# so the DVE queue is drained at quarter boundaries: tail_pre (recips + o_sb
# on DVE) then frees the av/sums psum banks immediately instead of ~9us late.
EXP_PAT_M0 = ['A', 'D', 'A', 'A', 'D', 'A', 'A', 'A']     # m0: DVE busy w/ tail_pre
EXP_PAT_EARLY = ['D', 'A', 'D', 'A', 'D', 'A', 'D', 'A']  # m 1..5: 4 DVE
EXP_PAT_LATE = ['D', 'A', 'D', 'A', 'D', 'A', 'A', 'A']   # m 6..7: 3 DVE, early

# ReduceScatter grouping: each collective freezes ALL DMA traffic for its
# full duration in the cost model (no DMA slice ever overlaps a collective),
# so fewer/larger collectives beat per-quarter ones despite the bigger tail.
# Groups must be consecutive quarters; a SINGLE end-of-kernel RS measured
# best (212.4us vs 215.9-221.1 for every split) since any mid-kernel
# collective freezes the XBAR transposes + ydram stores of the quarter it
# overlaps.
RS_GROUPS = [[int(c) for c in grp] for grp in
             os.environ.get("KRSG", "0123").split(",")]

_nc_cache = None
_last_result = None


def _build():
    nc = bacc.Bacc("TRN2", target_bir_lowering=False, debug=False, num_devices=CORES)

    xT_ext = nc.declare_dram_parameter("xT", [DIM, N], BF16, isOutput=False)
    xb_ext = nc.declare_dram_parameter("xb", [N, DIM], BF16, isOutput=False)
    wqkv_ext = nc.declare_dram_parameter("wqkv", [DIM, 772], BF16, isOutput=False)
    wout_ext = nc.declare_dram_parameter("wout", [2 * P, DIM], BF16, isOutput=False)
    cosP_ext = nc.declare_dram_parameter("cosP", [N, 512], BF16, isOutput=False)
    sinN_ext = nc.declare_dram_parameter("sinN", [N, 512], BF16, isOutput=False)
    bgn_ext = nc.declare_dram_parameter("bgn", [P, HL], F32, isOutput=False)
    out_ext = nc.declare_dram_parameter("out", [4 * P, DIM], BF16, isOutput=True)

    with tile.TileContext(nc) as tc:
        with (
            tc.tile_pool(name="wpool", bufs=1) as wpool,
            tc.tile_pool(name="persist", bufs=1) as persist,
            tc.tile_pool(name="xbp", bufs=8) as xbp,
            tc.tile_pool(name="sqp", bufs=4) as sqp,
            tc.tile_pool(name="small", bufs=8) as small,
            tc.tile_pool(name="tup", bufs=6) as tup,
            tc.tile_pool(name="qksp", bufs=4) as qksp,
            tc.tile_pool(name="ptp", bufs=14) as ptp,
            tc.tile_pool(name="op", bufs=2) as op_pool,
            tc.tile_pool(name="otp", bufs=2) as otp,
            tc.tile_pool(name="ysq", bufs=2) as ysq,
            tc.tile_pool(name="obp", bufs=2) as obp,
            # one 5-slot pool for all transient [128,512] psum tiles
            # (phase-A qk, phase-B scores s_t, tail out-proj y_ps): the
            # score->exp->free round trip (~1.4us) across 5 slots keeps PE
            # ahead of the exp pipeline; 3 slots paced the whole m-loop
            tc.tile_pool(name="ps_big", bufs=5, space="PSUM") as ps_big,
            tc.tile_pool(name="ps_va", bufs=2, space="PSUM") as ps_va,
            tc.tile_pool(name="dram", bufs=1, space="DRAM") as dramp,
        ):
            # ---- constants / weights (loads emitted later, ordered for
            # startup: xT/wqkv split across Pool+ACT, xb on SP) ----
            wqkv_sb = wpool.tile([P, KD, 772], BF16)
            wout_sb = wpool.tile([P, 2, DIM], BF16)
            bgn_sb = wpool.tile([P, HL], F32)
            cosP_sb = wpool.tile([P, TT, 512], BF16)
            sinN_sb = wpool.tile([P, TT, 512], BF16)
            zb = wpool.tile([P, 1], F32)
            nc.vector.memset(zb[:], 0.0)
            ones_mm = wpool.tile([P, 1], BF16)
            nc.vector.memset(ones_mm[:], 1.0)
            ones1 = wpool.tile([1, P], BF16)
            nc.vector.memset(ones1[:], 1.0)
            zrow = wpool.tile([1, 512], BF16)
            nc.vector.memset(zrow[:], 0.0)

            # ---- persistent activations ----
            xT_sb = persist.tile([P, KD, N], BF16)
            # qkT blocks: 0=q(h0,h1) 1=q(h2,h3) 2=k(h0,h1) 3=k(h2,h3); rows=dh
            qkT_sb = persist.tile([P, 4, N], BF16)
            v_sb = persist.tile([P, TT, 256], BF16)
            gates_sb = persist.tile([P, TT, HL], F32)
            ss_all = persist.tile([P, TT], F32)
            s_all = persist.tile([P, TT], F32)

            def emit_norm_group(g):
                """sum-sq (ACT Square+accum) + quake rsqrt for tiles 4g..4g+3.

                s = 32*ss^-0.5 with the /1024 folded into the quake seed's
                pre-scale; two Newton iterations on DVE. No act-table loads.
                """
                for tt in range(4 * g, 4 * g + 4):
                    xb_t = xbp.tile([P, DIM], BF16, name="xb_t")
                    nc.sync.dma_start(xb_t[:], xb_ext[tt * P:(tt + 1) * P, :])
                    scr = sqp.tile([P, DIM], BF16, name="scr")
                    nc.scalar.activation(scr[:], xb_t[:], AF.Square,
                                         accum_out=ss_all[:, tt:tt + 1])
                # quake-rsqrt Newton on DVE (Pool only supports TensorTensor/DMA)
                gs = slice(4 * g, 4 * g + 4)
                xp = small.tile([P, 4], F32, name="xp")
                nc.vector.tensor_scalar(out=xp[:], in0=ss_all[:, gs],
                                        scalar1=1.0 / 1024.0, scalar2=None,
                                        op0=ALU.mult)
                jt_ = small.tile([P, 4], I32, name="jt_")
                nc.vector.tensor_scalar(out=jt_[:], in0=xp[:].bitcast(I32),
                                        scalar1=1, scalar2=None,
                                        op0=ALU.logical_shift_right)
                y0b = small.tile([P, 4], I32, name="y0b")
                nc.vector.tensor_scalar(out=y0b[:], in0=jt_[:], scalar1=-1,
                                        scalar2=0x5f3759df, op0=ALU.mult,
                                        op1=ALU.add)
                cur = y0b[:].bitcast(F32)
                for it in range(2):
                    h_ = small.tile([P, 4], F32, name="h_")
                    nc.vector.tensor_tensor(out=h_[:], in0=cur, in1=cur,
                                            op=ALU.mult)
                    hx = small.tile([P, 4], F32, name="hx")
                    nc.vector.tensor_tensor(out=hx[:], in0=h_[:], in1=xp[:],
                                            op=ALU.mult)
                    w_ = small.tile([P, 4], F32, name="w_")
                    nc.vector.tensor_scalar(out=w_[:], in0=hx[:], scalar1=-0.5,
                                            scalar2=1.5, op0=ALU.mult,
                                            op1=ALU.add)
                    dst = s_all[:, gs] if it == 1 else \
                        small.tile([P, 4], F32, name="nx")[:]
                    nc.vector.tensor_tensor(out=dst, in0=cur, in1=w_[:],
                                            op=ALU.mult)
                    cur = dst

            def emit_tables_group(g):
                gs = slice(4 * g, 4 * g + 4)
                nc.sync.dma_start(
                    cosP_sb[:, gs, :],
                    cosP_ext.rearrange("(t p) f -> p t f", p=P)[:, gs, :])
                nc.gpsimd.dma_start(
                    sinN_sb[:, gs, :],
                    sinN_ext.rearrange("(t p) f -> p t f", p=P)[:, gs, :])

            # startup order: first xT sliver + wqkv halves (Pool/ACT, so the
            # ACT chunk lands before the squares), norm group 0 (xb on SP),
            # remaining xT, tables group 0, then per-group interleave
            xT_re = xT_ext.rearrange("(k p) t -> p k t", p=P)
            wq_re = wqkv_ext.rearrange("(k p) f -> p k f", p=P)
            nc.gpsimd.dma_start(xT_sb[:, :, 0:128], xT_re[:, :, 0:128])
            nc.gpsimd.dma_start(wqkv_sb[:, 0:2, :], wq_re[:, 0:2, :])
            nc.sync.dma_start(wqkv_sb[:, 2:4, :], wq_re[:, 2:4, :])
            nc.scalar.dma_start(wqkv_sb[:, 4:8, :], wq_re[:, 4:8, :])
            emit_norm_group(0)
            nc.gpsimd.dma_start(xT_sb[:, :, 128:512], xT_re[:, :, 128:512])
            emit_tables_group(0)
            nc.scalar.dma_start(bgn_sb[:], bgn_ext[:])
            # prefetch norm group 1 right away: rope(t4) needs s(g1), which
            # needs 4 squares serialized on ACT — a group emitted only at
            # tt=4 arrives ~5us late and stalls the qk psum slots
            emit_norm_group(1)
            emit_tables_group(1)
            for c in range(1, 4):
                nc.gpsimd.dma_start(
                    xT_sb[:, :, c * 512:(c + 1) * 512], xT_re[:, :, c * 512:(c + 1) * 512])

            # ---- phase A per token tile: QKV + RoPE + transposes + gates ----
            for tt in range(TT):
                if tt % 4 == 0 and 4 <= tt <= 8:
                    emit_norm_group(tt // 4 + 1)
                    emit_tables_group(tt // 4 + 1)
                s_ap = s_all[:, tt:tt + 1]
                # qk_ps gets the 3-slot pool (long rope-read lifetime), vg the
                # 2-slot one; both pools are otherwise idle during phase A
                qk_ps = ps_big.tile([P, 512], F32, name="qk_ps", tag="big")
                vg_ps = ps_va.tile([P, 260], F32, name="vg_ps", tag="va")
                for kd in range(KD):
                    lhsT = xT_sb[:, kd, tt * P:(tt + 1) * P]
                    nc.tensor.matmul(qk_ps[:], lhsT, wqkv_sb[:, kd, 0:512],
                                     start=(kd == 0), stop=(kd == KD - 1))
                    nc.tensor.matmul(vg_ps[:], lhsT, wqkv_sb[:, kd, 512:772],
                                     start=(kd == 0), stop=(kd == KD - 1))

                # rope with norm scale fused: qk' = (qk*s)*cos + (swap*s)*sin
                t_sb = tup.tile([P, 512], BF16, name="t_sb")
                nc.vector.scalar_tensor_tensor(
                    out=t_sb[:], in0=qk_ps[:], scalar=s_ap,
                    in1=cosP_sb[:, tt, :], op0=ALU.mult, op1=ALU.mult)
                u_sb = tup.tile([P, 512], BF16, name="u_sb")
                qkv8 = qk_ps[:].rearrange("p (b c) -> p b c", b=8)
                u8 = u_sb[:].rearrange("p (b c) -> p b c", b=8)
                sin8 = sinN_sb[:, tt, :].rearrange("p (b c) -> p b c", b=8)
                nc.vector.scalar_tensor_tensor(
                    out=u8[:, :, 0:32], in0=qkv8[:, :, 32:64], scalar=s_ap,
                    in1=sin8[:, :, 0:32], op0=ALU.mult, op1=ALU.mult)
                nc.vector.scalar_tensor_tensor(
                    out=u8[:, :, 32:64], in0=qkv8[:, :, 0:32], scalar=s_ap,
                    in1=sin8[:, :, 32:64], op0=ALU.mult, op1=ALU.mult)
                qk_sb = qksp.tile([P, 512], BF16, name="qk_sb")
                nc.gpsimd.tensor_tensor(out=qk_sb[:], in0=t_sb[:], in1=u_sb[:],
                                        op=ALU.add)
                nc.sync.dma_start_transpose(qkT_sb[:, :, tt * P:(tt + 1) * P], qk_sb[:])

                # v with norm scale (PSUM read: DVE)
                nc.vector.tensor_scalar(out=v_sb[:, tt, :], in0=vg_ps[:, 0:256],
                                        scalar1=s_ap, scalar2=None, op0=ALU.mult)

                # gates: sigmoid(s*z + b) via exp
                zt = small.tile([P, HL], BF16, name="zt")
                nc.vector.scalar_tensor_tensor(out=zt[:], in0=vg_ps[:, 256:260],
                                               scalar=s_ap, in1=bgn_sb[:],
                                               op0=ALU.mult, op1=ALU.add)
                ge = small.tile([P, HL], F32, name="ge")
                nc.scalar.activation(ge[:], zt[:], AF.Exp, scale=-1.0, bias=zb[:])
                gp = small.tile([P, HL], F32, name="gp")
                nc.vector.tensor_scalar_add(gp[:], ge[:], 1.0)
                nc.vector.reciprocal(gates_sb[:, tt, :], gp[:])

            nc.sync.dma_start(wout_sb[:], wout_ext.rearrange("(k p) f -> p k f", p=P))

            # ---- phase B: attention + out proj + per-quarter RS ----
            ydram_all = dramp.tile([4 * 512, DIM], BF16, name="ydall", tag="ydall")
            grp_obase = [sum(len(q) * P for q in RS_GROUPS[:i])
                         for i in range(len(RS_GROUPS))]
            rsout_g = [dramp.tile([len(Q) * P, DIM], BF16, name=f"rsg{i}",
                                  tag=f"rsg{i}")
                       for i, Q in enumerate(RS_GROUPS)]
            rs_after = {Q[-1]: gi for gi, Q in enumerate(RS_GROUPS)}

            def emit_rs(gi):
                Q = RS_GROUPS[gi]
                r0, r1 = Q[0] * 512, (Q[-1] + 1) * 512
                if not os.environ.get("KNOCOLL"):
                    nc.gpsimd.collective_compute(
                        "ReduceScatter", ALU.add,
                        replica_groups=REPLICA_GROUPS,
                        ins=[ydram_all[r0:r1, :].opt()],
                        outs=[rsout_g[gi][:].opt()],
                    )
                else:
                    nc.gpsimd.dma_start(
                        out_ext[grp_obase[gi]:grp_obase[gi] + len(Q) * P, :],
                        ydram_all[r0:r0 + len(Q) * P, :])

            # Schraudolph exp-to-bf16 bits: bits = s*0.125*log2e*128 + B
            SCH_A = float(0.125 * np.log2(np.e) * 128.0)
            SCH_B = float(16256.5 - 5.5)

            def emit_scores(ci, m):
                """scores+exp for jt pair (2m, 2m+1), all 4 heads."""
                pts = []
                for h in range(4):
                    pt = ptp.tile([P, 2, 512], BF16, name="pt")
                    for par in range(2):
                        jt = 2 * m + par
                        hh = 64 * (h % 2)
                        s_t = ps_big.tile([P, 512], F32, name="s_t", tag="big")
                        nc.tensor.matmul(
                            s_t[:],
                            qkT_sb[hh:hh + 64, 2 + h // 2, jt * P:(jt + 1) * P],
                            qkT_sb[hh:hh + 64, h // 2, ci * 512:(ci + 1) * 512],
                            start=True, stop=True,
                        )
                        pat = EXP_PAT_M0 if m == 0 else (
                            EXP_PAT_EARLY if m < 6 else EXP_PAT_LATE)
                        eng = pat[h * 2 + par]
                        if eng == 'A':
                            nc.scalar.activation(pt[:, par, :], s_t[:], AF.Exp,
                                                 scale=0.125, bias=zb[:])
                        else:
                            nc.vector.tensor_scalar(
                                out=pt[:, par, :].bitcast(U16),
                                in0=s_t[:], scalar1=SCH_A, scalar2=SCH_B,
                                op0=ALU.mult, op1=ALU.add)
                    pts.append(pt)
                return pts

            def emit_av(m, pts, av_t, sums_t):
                for h in range(4):
                    for qt in (0, 2, 1, 3):  # alternate psum banks
                        for par in range(2):
                            jt = 2 * m + par
                            nc.tensor.matmul(
                                av_t[qt][:, h * DH:(h + 1) * DH],
                                pts[h][:, par, qt * P:(qt + 1) * P],
                                v_sb[:, jt, h * DH:(h + 1) * DH],
                                start=False, stop=(m == TT // 2 - 1 and par == 1),
                                skip_group_check=True,
                            )
                            nc.tensor.matmul(
                                sums_t[:, (h * 4 + qt):(h * 4 + qt) + 1],
                                pts[h][:, par, qt * P:(qt + 1) * P],
                                ones_mm[:, 0:1],
                                start=False, stop=(m == TT // 2 - 1 and par == 1),
                                skip_group_check=True,
                            )

            def emit_tail_pre(ci, av_pair, sums_t):
                """normalize + gate + transpose; frees av/sums psum bufs.

                All four recips come first so the sums bank frees after
                ~0.8us (the next quarter's zero-matmul waits on it)."""
                av_t = [av_pair[qt // 2][:, qt % 2, :] for qt in range(4)]
                oT = otp.tile([P, 2, 512], BF16, name="oT")
                recs = []
                for qt in range(4):
                    rec = small.tile([P, HL], F32, name="rec")
                    nc.vector.reciprocal(
                        rec[:],
                        sums_t[:, 0:16].rearrange("p (h q) -> p q h", q=4)[:, qt, :])
                    recs.append(rec)
                for qt in range(4):
                    scl = small.tile([P, HL], F32, name="scl")
                    nc.vector.tensor_tensor(out=scl[:], in0=recs[qt][:],
                                            in1=gates_sb[:, ci * 4 + qt, :],
                                            op=ALU.mult)
                    o_sb = op_pool.tile([P, 256], BF16, name="o_sb")
                    scl_b = bass.AP(scl.tensor, scl.offset,
                                    [scl.ap[0], [1, 4], [0, DH]])
                    nc.vector.tensor_tensor(
                        out=o_sb[:].rearrange("p (h d) -> p h d", h=4),
                        in0=av_t[qt][:].rearrange("p (h d) -> p h d", h=4),
                        in1=scl_b, op=ALU.mult)
                    nc.sync.dma_start_transpose(oT[:, :, qt * P:(qt + 1) * P],
                                                o_sb[:])
                return oT

            def emit_y_qt(qt, oT, y_sbq):
                """out-projection + psum->sbuf copy for one 128-query block."""
                for oh in range(2):
                    y_ps = ps_big.tile([P, 512], F32, name="y_ps", tag="big")
                    for kh in range(2):
                        nc.tensor.matmul(
                            y_ps[:],
                            oT[:, kh, qt * P:(qt + 1) * P],
                            wout_sb[:, kh, oh * 512:(oh + 1) * 512],
                            start=(kh == 0), stop=(kh == 1),
                        )
                    dst = y_sbq[:, qt, oh * 512:(oh + 1) * 512]
                    if (qt + oh) % 2 == 0:
                        nc.scalar.activation(dst, y_ps[:], AF.Copy, bias=0.0)
                    else:
                        nc.vector.tensor_copy(dst, y_ps[:])

            def emit_tail_y_half(ci, oT, y_sbq, half):
                """out-projection + ydram store for 2 of 4 query blocks;
                splitting across two m's halves the y_ps psum-slot pressure
                that otherwise starves the score->exp pipeline mid-quarter.
                The group RS fires after the second half of its last
                quarter."""
                for qt in (2 * half, 2 * half + 1):
                    emit_y_qt(qt, oT, y_sbq)
                nc.sync.dma_start(
                    ydram_all[ci * 512:(ci + 1) * 512, :]
                    .rearrange("(q p) o -> p q o", p=P)
                    [:, 2 * half:2 * half + 2, :],
                    y_sbq[:, 2 * half:2 * half + 2, :])
                if half == 1 and ci in rs_after:
                    emit_rs(rs_after[ci])

            def emit_bounce(gi):
                """rsout -> out via SBUF, emitted at the very end of the
                program so its RS-completion wait never head-blocks a queue
                that still has time-critical work."""
                if os.environ.get("KNOCOLL"):
                    return
                Q = RS_GROUPS[gi]
                engs = [nc.gpsimd, nc.sync, nc.scalar]
                for k in range(len(Q)):
                    # rotate queues so the post-RS copies pipeline 3-wide
                    e = engs[k % 3]
                    bn = obp.tile([P, DIM], BF16, name="bn")
                    e.dma_start(bn[:], rsout_g[gi][k * P:(k + 1) * P, :])
                    e.dma_start(
                        out_ext[grp_obase[gi] + k * P:grp_obase[gi] + (k + 1) * P, :],
                        bn[:])

            def emit_tail_final(ci, av_pair, sums_t):
                """final quarter: per-qt pipeline (normalize -> transpose ->
                out-proj -> per-qt ydram store) so PE restarts ~1.5us after
                the last AV instead of waiting for the whole tail_pre, then
                one RS for the quarter."""
                av_t = [av_pair[qt // 2][:, qt % 2, :] for qt in range(4)]
                oT = otp.tile([P, 2, 512], BF16, name="oT")
                y_sbq = ysq.tile([P, 4, DIM], BF16, name="y_sbq")
                yd_re = ydram_all[ci * 512:(ci + 1) * 512, :].rearrange(
                    "(q p) o -> p q o", p=P)
                recs = []
                for qt in range(4):
                    rec = small.tile([P, HL], F32, name="rec")
                    nc.vector.reciprocal(
                        rec[:],
                        sums_t[:, 0:16].rearrange("p (h q) -> p q h", q=4)[:, qt, :])
                    recs.append(rec)
                for qt in range(4):
                    scl = small.tile([P, HL], F32, name="scl")
                    nc.vector.tensor_tensor(out=scl[:], in0=recs[qt][:],
                                            in1=gates_sb[:, ci * 4 + qt, :],
                                            op=ALU.mult)
                    o_sb = op_pool.tile([P, 256], BF16, name="o_sb")
                    scl_b = bass.AP(scl.tensor, scl.offset,
                                    [scl.ap[0], [1, 4], [0, DH]])
                    nc.vector.tensor_tensor(
                        out=o_sb[:].rearrange("p (h d) -> p h d", h=4),
                        in0=av_t[qt][:].rearrange("p (h d) -> p h d", h=4),
                        in1=scl_b, op=ALU.mult)
                    nc.sync.dma_start_transpose(oT[:, :, qt * P:(qt + 1) * P],
                                                o_sb[:])
                    emit_y_qt(qt, oT, y_sbq)
                    e = nc.sync if qt % 2 == 0 else nc.gpsimd
                    e.dma_start(yd_re[:, qt, :], y_sbq[:, qt, :])
                emit_rs(rs_after[ci])

            pending = None  # (ci, av_pair, sums_t) awaiting tail emission
            bounce_pending = None  # quarter whose RS is in flight, bounce due
            oT_prev = None
            for ci in range(NQ):
                if pending is not None:
                    oT_prev = emit_tail_pre(*pending)
                av_pair = [ps_va.tile([P, 2, 256], F32, name=f"av{i}", tag="va")
                           for i in range(2)]
                av_t = [av_pair[qt // 2][:, qt % 2, :] for qt in range(4)]
                sums_t = ps_big.tile([P, 16], F32, name="sums_t", tag="sums", bufs=1)
                # AV lags two m's behind scores: av(m) is emitted after
                # scores(m+2), so every pt has ~2 m-periods (~6us) of exp
                # slack and PE never waits on stragglers, even across the
                # quarter boundary where tail_pre occupies DVE first.
                pts_q = []
                for m in range(TT // 2):
                    pts_q.append(emit_scores(ci, m))
                    if m == 1:
                        # zero accumulator banks via K=1 zero matmuls (see
                        # docstring); emitted ~2 m-periods into the quarter
                        # so tail_pre has long freed the previous banks
                        for i in range(2):
                            nc.tensor.matmul(av_pair[i][:], ones1[:], zrow[:],
                                             start=True, stop=True)
                        nc.tensor.matmul(sums_t[:], ones1[:], zrow[:, 0:16],
                                         start=True, stop=True)
                    if m >= 2:
                        emit_av(m - 2, pts_q[m - 2], av_t, sums_t)
                    if m == 1 and pending is not None:
                        y_sbq_prev = ysq.tile([P, 4, DIM], BF16, name="y_sbq")
                        emit_tail_y_half(pending[0], oT_prev, y_sbq_prev, 0)
                    if m == 3 and pending is not None:
                        emit_tail_y_half(pending[0], oT_prev, y_sbq_prev, 1)
                        bounce_pending = pending[0]
                        pending = None
                emit_av(TT // 2 - 2, pts_q[TT // 2 - 2], av_t, sums_t)
                emit_av(TT // 2 - 1, pts_q[TT // 2 - 1], av_t, sums_t)
                pending = (ci, av_pair, sums_t)

            emit_tail_final(*pending)
            for gi in range(len(RS_GROUPS)):
                emit_bounce(gi)

    nc.compile()
    return nc


def _get_nc():
    global _nc_cache
    if _nc_cache is None:
        _nc_cache = _build()
    return _nc_cache


_PERM_EO = np.concatenate([np.arange(0, DH, 2), np.arange(1, DH, 2)])


def _shard(core, x, rotary_cos, rotary_sin, gamma, w_qkv, w_gates, b_gates, w_out):
    g, r = core // 4, core % 4
    heads = np.arange(4 * r, 4 * r + 4)
    wq = w_qkv[0 * DIM:1 * DIM] * gamma[None, :]
    wk = w_qkv[1 * DIM:2 * DIM] * gamma[None, :]
    wv = w_qkv[2 * DIM:3 * DIM]

    def qk_rows(w):
        idx = (heads[:, None] * DH + _PERM_EO[None, :]).reshape(-1)
        return w[idx]

    v_rows = wv[(heads[:, None] * DH + np.arange(DH)[None, :]).reshape(-1)]
    wg_rows = w_gates[heads] * gamma[None, :]
    wqkv_t = np.concatenate([qk_rows(wq), qk_rows(wk), v_rows, wg_rows],
                            axis=0).T
    wout_t = w_out[:, heads[0] * DH:heads[0] * DH + HL * DH].T

    cos = rotary_cos[0, 0]  # (N, DH)
    sin = rotary_sin[0, 0]
    cosP = np.tile(np.concatenate([cos[:, 0::2], cos[:, 1::2]], axis=1), (1, 8))
    sinN = np.tile(np.concatenate([-sin[:, 0::2], sin[:, 1::2]], axis=1), (1, 8))

    bf = ml_dtypes.bfloat16
    return {
        "xT": np.ascontiguousarray(x[g].T).astype(bf),
        "xb": np.ascontiguousarray(x[g]).astype(bf),
        "wqkv": np.ascontiguousarray(wqkv_t).astype(bf),
        "wout": np.ascontiguousarray(wout_t).astype(bf),
        "cosP": np.ascontiguousarray(cosP).astype(bf),
        "sinN": np.ascontiguousarray(sinN).astype(bf),
        "bgn": np.tile(b_gates[heads][None, :], (P, 1)).astype(np.float32),
    }


def kernel(x, rotary_cos, rotary_sin, gamma, w_qkv, w_gates, b_gates, w_out):
    global _last_result
    args = [np.asarray(a, np.float32) for a in
            (x, rotary_cos, rotary_sin, gamma, w_qkv, w_gates, b_gates, w_out)]
    nc = _get_nc()
    in_maps = [_shard(c, *args) for c in range(CORES)]
    try:
        res = run_bass_kernel_spmd(
            nc, in_maps, core_ids=list(range(CORES)),
            trace=bool(os.environ.get("KTRACE")),
        )
    except ModuleNotFoundError:
        res = run_bass_kernel_spmd(nc, in_maps, core_ids=list(range(CORES)))
    _last_result = res
    full = np.zeros((B, N, DIM), np.float32)
    for c in range(CORES):
        g, r = c // 4, c % 4
        o = np.asarray(res.results[c]["out"]).astype(np.float32)
        base = 0
        for Q in RS_GROUPS:
            ow = len(Q) * P
            qs = Q[0] * 512
            full[g, qs + r * ow:qs + (r + 1) * ow, :] = o[base:base + ow]
            base += ow
    return full
